# revision 24
# baseline (speedup 1.0000x reference)
"""MGDPR (gnn_message_passing) Trainium2 kernel, 8 NeuronCores.

Sharding: nodes row-sharded 4-way within each batch element; cores 0-3 own
batch 0, cores 4-7 own batch 1 (375 dest nodes each). Source nodes live in a
padded space (384 per shard = 3x128 tiles) so gather shards align with
128-partition tiles. All per-node tensors are channel-major on chip
([C, nodes]); the diffusion matmul contracts source nodes on partitions using
node-major h tiles produced by PE transposes of the (channel-major) gathered
h. h is re-gathered across the 4 cores of each batch after layers 0 and 1 via
a channel-major AllGather (no pre-transpose needed). No 8-core barrier: the
4-core AllGathers absorb launch skew within each group only. All matmuls are
bf16 (f32r matmuls run in slow fp32-HIGH mode on hw) except the GroupNorm
stats which stay f32r for variance accuracy. h_prime never depends on node
data (zeros init + per-channel affine), so it folds into a per-layer bias.
"""

import numpy as np

try:
    import concourse.bass as bass
except ImportError:
    import sys

    sys.path.insert(0, "/opt/trn_rl_repo")
    import concourse.bass as bass

import concourse.mybir as mybir
import concourse.tile as tile
from concourse import bacc
from concourse.bass_utils import run_bass_kernel_spmd

B, N, T, DIN, C, R, K, L, H, OUT = 2, 1500, 20, 32, 128, 5, 5, 3, 4, 2
HD = C // H
EPS = 1e-5
NCORES = 8
NS = N // 4          # 375 real nodes per shard
NSP = NS + 1         # dest cols per core (padded even)
NSH = 384            # padded source nodes per shard (3x128)
NT = 12              # source tiles (4*384/128)
MPAD = 4 * NSH
CW = NSP // 2        # chunk width (188)
RG = [[0, 1, 2, 3], [4, 5, 6, 7]]
F32R = mybir.dt.float32r
F32 = mybir.dt.float32
BF16 = mybir.dt.bfloat16
F8 = mybir.dt.float8e4
AF = mybir.ActivationFunctionType

_NC_CACHE = {}


def _build_nc():
    if "nc" in _NC_CACHE:
        return _NC_CACHE["nc"]
    nc = bacc.Bacc(None, target_bir_lowering=False, debug=False, num_devices=NCORES)

    # ---- per-core inputs ----
    adjt = nc.dram_tensor("adjt", [NT, 128, R, NSP], F8, kind="ExternalInput")
    xt = nc.dram_tensor("xt", [DIN + 1, MPAD], BF16, kind="ExternalInput")
    # ---- replicated consts (host-prelaid in SBUF layout, partition-first) ----
    wp_d = nc.dram_tensor("wp", [C, L * R * C], BF16, kind="ExternalInput")
    qkvo_d = nc.dram_tensor("qkvo", [C, L * 4 * C], BF16, kind="ExternalInput")
    w2at_d = nc.dram_tensor("w2at", [C, L * C], BF16, kind="ExternalInput")
    ow1t_d = nc.dram_tensor("ow1t", [C, C], F8, kind="ExternalInput")
    ow2t_d = nc.dram_tensor("ow2t", [C, OUT], BF16, kind="ExternalInput")
    embt_d = nc.dram_tensor("embt", [DIN + 1, C], BF16, kind="ExternalInput")
    mh_d = nc.dram_tensor("mh", [C, C], BF16, kind="ExternalInput")
    mmu_d = nc.dram_tensor("mmu", [C, C], F32R, kind="ExternalInput")
    ident_d = nc.dram_tensor("ident", [C, C], BF16, kind="ExternalInput")
    ident8_d = nc.dram_tensor("ident8", [C, C], F8, kind="ExternalInput")
    cols_d = nc.dram_tensor("cols", [C, 4 * L + 3], F32, kind="ExternalInput")
    cols2_d = nc.dram_tensor("cols2", [C, 4 * L], F32, kind="ExternalInput")
    rows_d = nc.dram_tensor("rows", [1, 4 * L * C], BF16, kind="ExternalInput")

    outt = nc.dram_tensor("outt", [OUT, NSP], F32R, kind="ExternalOutput")

    # gather buffers (internal DRAM), channel-major [C, padded shard nodes]
    g_in = [nc.dram_tensor(f"g_in_{l}", [C, NSH], F8) for l in range(2)]
    g_out = [nc.dram_tensor(f"g_out_{l}", [4 * C, NSH], F8) for l in range(2)]
    gw_in = nc.dram_tensor("gw_in", [C, 8], F8)
    gw_out = nc.dram_tensor("gw_out", [4 * C, 8], F8)

    with tile.TileContext(nc) as tc:
        with (
            tc.tile_pool(name="persist", bufs=1) as pers,
            tc.tile_pool(name="ret", bufs=2) as ret,
            tc.tile_pool(name="zwork", bufs=6) as zwork,
            tc.tile_pool(name="big", bufs=2) as big,
            tc.tile_pool(name="misc", bufs=2) as misc,
            tc.tile_pool(name="pz", bufs=3, space="PSUM") as pz,
            tc.tile_pool(name="pm", bufs=2, space="PSUM") as pm,
            tc.tile_pool(name="pp", bufs=3, space="PSUM") as pp,
        ):
            # ---------- resident tensors ----------
            adjsb = pers.tile([128, NT, R, NSP], F8, tag="adjsb")
            hnat = pers.tile([128, NT * 128], F8, tag="hnat")
            hsh = pers.tile([128, 4 * NSH], F8, tag="hsh")
            xtsb = pers.tile([DIN + 1, MPAD], BF16, tag="xtsb")
            embtsb = pers.tile([DIN + 1, C], BF16, tag="embtsb")
            wpsb = pers.tile([C, L * R * C], BF16, tag="wpsb")
            qkvosb = pers.tile([C, L * 4 * C], BF16, tag="qkvosb")
            w2atsb = pers.tile([C, L * C], BF16, tag="w2atsb")
            ow1tsb = pers.tile([C, C], F8, tag="ow1tsb")
            ow2tsb = pers.tile([C, OUT], BF16, tag="ow2tsb")
            mhsb = pers.tile([C, C], BF16, tag="mhsb")
            mmusb = pers.tile([C, C], F32R, tag="mmusb")
            identsb = pers.tile([C, C], BF16, tag="identsb")
            ident8sb = pers.tile([C, C], F8, tag="ident8sb")
            colsb = pers.tile([C, 4 * L + 3], F32, tag="colsb")
            colsb2 = pers.tile([C, 4 * L], F32, tag="colsb2")
            rowsb = pers.tile([1, 4 * L * C], BF16, tag="rowsb")
            onesb = pers.tile([1, NSP], BF16, tag="onesb")
            zerosb = pers.tile([128, NSH], F8, tag="zerosb")

            # ---------- input DMA, priority-ordered across 3 queues ----------
            # sync HWDGE: embed inputs then even adj tiles
            nc.sync.dma_start(xtsb[:], xt[:, :])
            nc.sync.dma_start(embtsb[:], embt_d[:, :])
            # scalar HWDGE: wp (needed first at L0 proj) then odd adj tiles
            nc.scalar.dma_start(wpsb[:], wp_d[:, :])
            nc.sync.dma_start(adjsb[:, 0:6:2, :, :], adjt[0:6:2])
            nc.scalar.dma_start(adjsb[:, 1:6:2, :, :], adjt[1:6:2])
            nc.sync.dma_start(adjsb[:, 6:12:2, :, :], adjt[6:12:2])
            nc.scalar.dma_start(adjsb[:, 7:12:2, :, :], adjt[7:12:2])
            # gpsimd SWDGE: the rest, roughly in order of first use
            nc.gpsimd.dma_start(qkvosb[:], qkvo_d[:, :])
            nc.gpsimd.dma_start(mhsb[:], mh_d[:, :])
            nc.gpsimd.dma_start(mmusb[:], mmu_d[:, :])
            nc.gpsimd.dma_start(w2atsb[:], w2at_d[:, :])
            nc.gpsimd.dma_start(identsb[:], ident_d[:, :])
            nc.gpsimd.dma_start(ident8sb[:], ident8_d[:, :])
            nc.gpsimd.dma_start(colsb[:], cols_d[:, :])
            nc.gpsimd.dma_start(colsb2[:], cols2_d[:, :])
            nc.gpsimd.dma_start(rowsb[:], rows_d[:, :])
            nc.gpsimd.dma_start(ow1tsb[:], ow1t_d[:, :])
            nc.gpsimd.dma_start(ow2tsb[:], ow2t_d[:, :])

            nc.vector.memset(onesb[:], 1.0)
            nc.vector.memset(zerosb[:], 0.0)
            # tiny AllGather fired immediately: pays the first-collective
            # setup + absorbs launch skew while the adj DMA streams in
            nc.sync.dma_start(gw_in[:, :], zerosb[:, 0:8])
            nc.gpsimd.collective_compute(
                "AllGather", mybir.AluOpType.bypass,
                replica_groups=RG,
                ins=[gw_in[:, :].opt()],
                outs=[gw_out[:, :].opt()],
            )
            # pre-zero the gather inputs so pad cols (and the NSP pad col's
            # neighbors) are exact zeros, not junk DRAM that could be NaN
            nc.gpsimd.dma_start(g_in[0][:, :], zerosb[:])
            nc.gpsimd.dma_start(g_in[1][:, :], zerosb[:])

            def col(i):
                return colsb[:, i : i + 1]

            def row(l, j):
                return rowsb[0:1, (4 * l + j) * C : (4 * l + j + 1) * C]

            wp3 = wpsb.rearrange("p (l r co) -> p l r co", l=L, r=R)
            qk4 = qkvosb.rearrange("p (l i co) -> p l i co", l=L, i=4)
            w2a3 = w2atsb.rearrange("p (l co) -> p l co", l=L)

            def blip(src):
                """Tiny junk matmul chained off a chain tile: keeps the PE's
                HAM activity window busy through DVE/ACT-only stretches so the
                clock gate stays at 2.4 GHz. Output is never read."""
                jp = pz.tile([128, 8], F32, name="blip", tag="zs")
                nc.tensor.matmul(jp[:], identsb[:], src[:, 0:8],
                                 start=True, stop=True, skip_group_check=True)

            copy_eng = [0]

            def copy_alt(dst, src):
                if copy_eng[0] % 2 == 0:
                    nc.vector.tensor_copy(dst, src)
                else:
                    nc.scalar.copy(dst, src)
                copy_eng[0] += 1

            # ---------- h0 = embedding (node-major, source-node space) ----------
            for mt in range(NT):
                ep = pp.tile([128, 128], F32, tag="ps")
                nc.tensor.matmul(
                    ep[:], xtsb[:, mt * 128 : (mt + 1) * 128], embtsb[:],
                    start=True, stop=True,
                )
                copy_alt(hnat[:, mt * 128 : (mt + 1) * 128], ep[:])

            # ---------- layer machinery ----------
            CHUNKS = [(0, CW), (CW, NSP - CW)]
            RGROUPS = [(0, 1, 2), (3, 4)]

            def diff_mms(rg, c0, cw):
                """Emit the adjacency matmuls for one r-group; returns zps."""
                zps = {r: pz.tile([128, cw], F32, name=f"zps{r}", tag="zs") for r in rg}
                for mt in range(NT):
                    for r in rg:
                        nc.tensor.matmul(
                            zps[r][:],
                            hnat[:, mt * 128 : (mt + 1) * 128],
                            adjsb[:, mt, r, c0 : c0 + cw],
                            start=(mt == 0), stop=(mt == NT - 1),
                            skip_group_check=True,
                        )
                return zps

            def diff_proj(l, rg, zps, mps, cw):
                for r in rg:
                    zsb = zwork.tile([128, cw], BF16, tag="zsb")
                    copy_alt(zsb[:], zps[r][:])
                    nc.tensor.matmul(
                        mps[:], wp3[:, l, r, :], zsb[:],
                        start=(r == 0), stop=(r == R - 1),
                        skip_group_check=True,
                    )

            def diffusion(l, c0, cw):
                """Returns mps PSUM tile [128, cw] with merged diffusion."""
                mps = pm.tile([128, cw], F32, tag="mps")
                for rg in RGROUPS:
                    zps = diff_mms(rg, c0, cw)
                    diff_proj(l, rg, zps, mps, cw)
                return mps

            def ret_head(l, mps, c0, cw):
                """relu + q/k/v projections. Returns (qps, ksb, vsb)."""
                hdT = ret.tile([128, cw], BF16, tag="hdT")
                nc.scalar.activation(
                    hdT[:], mps[:], AF.Relu, bias=col(4 * l + 0), scale=1.0
                )
                qps = pp.tile([128, cw], F32, tag="ps")
                nc.tensor.matmul(qps[:], qk4[:, l, 0, :], hdT[:],
                                 start=True, stop=False, skip_group_check=True)
                nc.tensor.matmul(qps[:], row(l, 0), onesb[0:1, c0 : c0 + cw],
                                 start=False, stop=True, skip_group_check=True)
                kps = pp.tile([128, cw], F32, tag="ps")
                nc.tensor.matmul(kps[:], qk4[:, l, 1, :], hdT[:],
                                 start=True, stop=True)
                ksb = ret.tile([128, cw], F32R, tag="ksb")
                nc.scalar.activation(ksb[:], kps[:], AF.Identity, bias=row_as_col(l, 1))
                vps = pp.tile([128, cw], F32, tag="ps")
                nc.tensor.matmul(vps[:], qk4[:, l, 2, :], hdT[:],
                                 start=True, stop=True)
                vsb = ret.tile([128, cw], F32R, tag="vsb")
                nc.vector.tensor_scalar_add(vsb[:], vps[:], row_as_col(l, 2))
                return qps, ksb, vsb

            def row_as_col(l, j):
                # kb/vb applied as per-partition activation-bias columns
                return colsb2[:, (4 * l + j) : (4 * l + j) + 1]

            def ret_tail(l, qps, ksb, vsb, c0, cw, hnT_full):
                """Retention tail as 4 emission segments so the caller can
                interleave them with another chunk's diffusion matmuls (engine
                streams execute in emission order)."""
                st = {}

                def seg1():
                    st["qk"] = ret.tile([128, cw], BF16, name="qk", tag="qk")
                    nc.vector.tensor_mul(st["qk"][:], ksb[:], qps[:])
                    st["sbps"] = pp.tile([128, cw], F32, name="sbps", tag="ps")
                    nc.tensor.matmul(st["sbps"][:], mhsb[:], st["qk"][:],
                                     start=True, stop=True)
                    st["osb"] = ret.tile([128, cw], BF16, name="osb", tag="osb")
                    nc.vector.tensor_mul(st["osb"][:], vsb[:], st["sbps"][:])
                    blip(st["qk"])

                def seg2():
                    st["o2ps"] = pp.tile([128, cw], F32, name="o2ps", tag="ps")
                    nc.tensor.matmul(st["o2ps"][:], qk4[:, l, 3, :], st["osb"][:],
                                     start=True, stop=False, skip_group_check=True)
                    nc.tensor.matmul(st["o2ps"][:], row(l, 3),
                                     onesb[0:1, c0 : c0 + cw],
                                     start=False, stop=True, skip_group_check=True)
                    st["sq"] = ret.tile([128, cw], F32R, name="sq", tag="sq")
                    nc.scalar.activation(st["sq"][:], st["o2ps"][:], AF.Square)
                    st["o2sb"] = ret.tile([128, cw], F32R, name="o2sb", tag="o2sb")
                    nc.vector.tensor_copy(st["o2sb"][:], st["o2ps"][:])

                def seg3():
                    mups = pp.tile([128, cw], F32, tag="ps")
                    nc.tensor.matmul(mups[:], mmusb[:], st["o2sb"][:],
                                     start=True, stop=True)
                    msps = pp.tile([128, cw], F32, tag="ps")
                    nc.tensor.matmul(msps[:], mmusb[:], st["sq"][:],
                                     start=True, stop=True)
                    mu2 = ret.tile([128, cw], F32R, tag="mu2")
                    nc.scalar.activation(mu2[:], mups[:], AF.Square)
                    tsb = ret.tile([128, cw], BF16, tag="tsb")
                    nc.vector.tensor_sub(tsb[:], st["o2sb"][:], mups[:])
                    varsb = ret.tile([128, cw], F32R, tag="varsb")
                    nc.vector.tensor_sub(varsb[:], msps[:], mu2[:])
                    rstd = ret.tile([128, cw], BF16, tag="rstd")
                    # 1/sqrt(var+eps) in one table-resident activation; the
                    # abs is a no-op since var+eps > 0
                    nc.scalar.activation(rstd[:], varsb[:],
                                         AF.Abs_reciprocal_sqrt, bias=col(4 * L))
                    blip(tsb)
                    # hr = (o2-mu)*rstd*gn_g + gn_b; the gn_b term is folded
                    # into the w2 bias on the host, so one stt does the rest
                    st["hrT"] = ret.tile([128, cw], BF16, name="hrT", tag="hrT")
                    nc.vector.scalar_tensor_tensor(
                        st["hrT"][:], tsb[:], col(4 * l + 2), rstd[:],
                        mybir.AluOpType.mult, mybir.AluOpType.mult,
                    )

                def seg4():
                    h2ps = pp.tile([128, cw], F32, tag="ps")
                    nc.tensor.matmul(h2ps[:], w2a3[:, l, :], st["hrT"][:],
                                     start=True, stop=True)
                    nc.scalar.activation(
                        hnT_full[:, c0 : c0 + cw], h2ps[:], AF.Relu,
                        bias=col(4 * l + 1), scale=1.0,
                    )

                return seg1, seg2, seg3, seg4

            # ---------- layers ----------
            for l in range(L):
                if l > 0:
                    # rebuild hnat from the gathered channel-major h
                    for s, eng in enumerate((nc.sync, nc.scalar, nc.gpsimd,
                                             nc.sync)):
                        eng.dma_start(
                            hsh[:, s * NSH : (s + 1) * NSH],
                            g_out[l - 1][s * 128 : (s + 1) * 128, :],
                        )
                    for t in range(NT):
                        # fp8 PE transpose requires output element step 2
                        tp = pp.tile([128, 256], F8, tag="ps")
                        nc.tensor.transpose(
                            tp[:, 0:256:2], hsh[:, t * 128 : (t + 1) * 128],
                            ident8sb[:],
                        )
                        copy_alt(hnat[:, t * 128 : (t + 1) * 128], tp[:, 0:256:2])

                hnT_full = big.tile([C, NSP], F8, tag="hnT")
                (a0, aw), (b0, bw) = CHUNKS

                mpsA = diffusion(l, a0, aw)
                qA = ret_head(l, mpsA, a0, aw)
                # chunk B diffusion interleaved with chunk A retention tail:
                # A's DVE/ACT chain runs while the PE grinds B's adjacency
                # matmuls; A's few PE hops slot between B's r-groups.
                s1, s2, s3, s4 = ret_tail(l, *qA, a0, aw, hnT_full)
                mpsB = pm.tile([128, bw], F32, tag="mps")
                zB1 = diff_mms(RGROUPS[0], b0, bw)
                s1()
                diff_proj(l, RGROUPS[0], zB1, mpsB, bw)
                s2()
                zB2 = diff_mms(RGROUPS[1], b0, bw)
                s3()
                diff_proj(l, RGROUPS[1], zB2, mpsB, bw)
                s4()
                qB = ret_head(l, mpsB, b0, bw)
                t1, t2, t3, t4 = ret_tail(l, *qB, b0, bw, hnT_full)
                t1(); t2(); t3(); t4()

                if l < 2:
                    nc.sync.dma_start(g_in[l][:, 0:NSP], hnT_full[:, 0:NSP])
                    nc.gpsimd.collective_compute(
                        "AllGather", mybir.AluOpType.bypass,
                        replica_groups=RG,
                        ins=[g_in[l][:, :].opt()],
                        outs=[g_out[l][:, :].opt()],
                    )
                    # self-paced junk mm/copy hops to keep the PE clock gate
                    # warm across the collective wait (~1us per hop)
                    cur = hnT_full
                    for i in range(5):
                        jp = pz.tile([128, 8], F32, name=f"wc{i}", tag="zs")
                        nc.tensor.matmul(jp[:], identsb[:], cur[:, 0:8],
                                         start=True, stop=True,
                                         skip_group_check=True)
                        js = zwork.tile([128, 8], BF16, name=f"wcs{i}", tag="zsb")
                        nc.scalar.copy(js[:], jp[:])
                        cur = js
                else:
                    # final head
                    hmps = pp.tile([128, NSP], F32, tag="ps")
                    nc.tensor.matmul(hmps[:], ow1tsb[:], hnT_full[:],
                                     start=True, stop=True)
                    hmsb = misc.tile([C, NSP], BF16, tag="hmsb")
                    nc.scalar.activation(
                        hmsb[:], hmps[:], AF.Relu, bias=col(4 * L + 1)
                    )
                    oops = pp.tile([OUT, NSP], F32, tag="ps")
                    nc.tensor.matmul(oops[:], ow2tsb[:], hmsb[:],
                                     start=True, stop=True)
                    oosb = misc.tile([OUT, NSP], F32R, tag="oosb")
                    nc.scalar.activation(
                        oosb[:], oops[:], AF.Identity,
                        bias=colsb[0:OUT, 4 * L + 2 : 4 * L + 3],
                    )
                    nc.sync.dma_start(outt[:, :], oosb[:])

    nc.finalize()
    _NC_CACHE["nc"] = nc
    return nc


def _prep(inputs):
    import ml_dtypes

    bf16 = ml_dtypes.bfloat16
    f8 = ml_dtypes.float8_e4m3
    f32 = np.float32

    def g(name):
        return np.asarray(inputs[name], f32)

    x, adj = g("x"), g("adj_list")
    alpha, transition = g("alpha"), g("transition")
    conv_w, conv_b = g("conv_w"), g("conv_b")
    w1, b1, eb1 = g("w1"), g("b1"), g("eb1")
    w2, b2, eb2 = g("w2"), g("b2"), g("eb2")

    a = alpha - alpha.max(-1, keepdims=True)
    e = np.exp(a)
    srow = (e / e.sum(-1, keepdims=True)).sum(-1)          # [L,R]
    Wm = transition.mean(axis=2)                            # [L,R,C,C]
    Wp = (conv_w * srow)[:, :, None, None] * np.swapaxes(Wm, -1, -2)

    hp = np.zeros((C,), f32)
    b2eff = np.zeros((L, C), f32)
    for l in range(L):
        # gn_b's contribution through w2 is folded in here so the kernel's
        # GN affine is a single (x-mu)*rstd*gn_g op
        b2eff[l] = b2[l] + eb2[l] + w2[l][:, C:] @ hp + w2[l][:, :C] @ g("gn_b")[l]
        hp = np.maximum(hp @ w1[l].T + b1[l] + eb1[l], 0.0).astype(f32)

    qkvo = np.stack(
        [np.swapaxes(g(w), -1, -2) for w in ("qw", "kw", "vw", "ow")], axis=1
    )  # [L,4,C,C] in lhsT layout

    hid = np.arange(C) // HD
    same = (hid[:, None] == hid[None, :]).astype(f32)       # [C,C]

    cols = np.zeros((C, 4 * L + 3), f32)
    cols2 = np.zeros((C, 4 * L), f32)
    rows = np.zeros((1, 4 * L * C), f32)
    for l in range(L):
        cols[:, 4 * l + 0] = conv_b[l]
        cols[:, 4 * l + 1] = b2eff[l]
        cols[:, 4 * l + 2] = g("gn_g")[l]
        cols[:, 4 * l + 3] = g("gn_b")[l]
        for j, nm in enumerate(("qb", "kb", "vb", "ob")):
            cols2[:, 4 * l + j] = g(nm)[l]
            rows[0, (4 * l + j) * C : (4 * l + j + 1) * C] = g(nm)[l]
    cols[:, 4 * L] = EPS
    cols[:, 4 * L + 1] = g("out_b1")
    cols[:OUT, 4 * L + 2] = g("out_b2")

    consts = {
        "wp": np.ascontiguousarray(
            (Wp / 16.0).transpose(2, 0, 1, 3).reshape(C, L * R * C)
        ).astype(bf16),
        "qkvo": np.ascontiguousarray(
            qkvo.transpose(2, 0, 1, 3).reshape(C, L * 4 * C)
        ).astype(bf16),
        "w2at": np.ascontiguousarray(
            np.swapaxes(w2[:, :, :C], -1, -2).transpose(1, 0, 2).reshape(C, L * C)
        ).astype(bf16),
        "ow1t": np.ascontiguousarray(g("out_w1").T).astype(f8),
        "ow2t": np.ascontiguousarray(g("out_w2").T).astype(bf16),
        "embt": np.concatenate(
            [g("emb_w").T, g("emb_b")[None, :]], axis=0
        ).astype(bf16),
        "mh": same.astype(bf16),
        "mmu": (same / HD).astype(f32),
        "ident": np.eye(C, dtype=f32).astype(bf16),
        "ident8": np.eye(C, dtype=f32).astype(f8),
        "cols": cols,
        "cols2": cols2,
        "rows": rows.astype(bf16),
    }

    xlast = x[:, :, -1, :]                                   # [B,N,DIN]
    in_maps = []
    for k in range(NCORES):
        b, s = k // 4, k % 4
        asub = adj[b][:, s * NS : (s + 1) * NS, :]           # [R,NS,N] dest rows
        ap = np.zeros((R, NSP, MPAD), f32)
        for s2 in range(4):
            ap[:, :NS, s2 * NSH : s2 * NSH + NS] = asub[:, :, s2 * NS : (s2 + 1) * NS]
        a3 = ap.transpose(2, 0, 1).reshape(NT, 128, R, NSP)  # [mt, mi, R, NSP]
        xt = np.zeros((DIN + 1, MPAD), f32)
        for s2 in range(4):
            xt[:DIN, s2 * NSH : s2 * NSH + NS] = xlast[b, s2 * NS : (s2 + 1) * NS].T
        xt[DIN, :] = 1.0
        in_maps.append(
            dict(consts, adjt=(np.ascontiguousarray(a3) * 16.0).astype(f8),
                 xt=xt.astype(bf16))
        )
    return in_maps


def kernel(**inputs):
    nc = _build_nc()
    in_maps = _prep(inputs)
    res = run_bass_kernel_spmd(nc, in_maps, core_ids=list(range(NCORES)))
    out = np.zeros((B, N, OUT), np.float32)
    for k in range(NCORES):
        b, s = k // 4, k % 4
        out[b, s * NS : (s + 1) * NS, :] = res.results[k]["outt"][:, :NS].T
    return out


# revision 25
# speedup vs baseline: 1.6876x; 1.6876x over previous
"""MGDPR (gnn_message_passing) Trainium2 kernel, 8 NeuronCores.

Sharding: nodes row-sharded 4-way within each batch element; cores 0-3 own
batch 0, cores 4-7 own batch 1 (375 dest nodes each). Source nodes live in a
padded space (384 per shard = 3x128 tiles) so gather shards align with
128-partition tiles. All per-node tensors are channel-major on chip
([C, nodes]); the diffusion matmul contracts source nodes on partitions using
node-major h tiles produced by PE transposes of the (channel-major) gathered
h. h is re-gathered across the 4 cores of each batch after layers 0 and 1 via
a channel-major AllGather (no pre-transpose needed). No 8-core barrier: the
4-core AllGathers absorb launch skew within each group only. All matmuls are
bf16 (f32r matmuls run in slow fp32-HIGH mode on hw) except the GroupNorm
stats which stay f32r for variance accuracy. h_prime never depends on node
data (zeros init + per-channel affine), so it folds into a per-layer bias.
"""

import numpy as np

try:
    import concourse.bass as bass
except ImportError:
    import sys

    sys.path.insert(0, "/opt/trn_rl_repo")
    import concourse.bass as bass

import concourse.mybir as mybir
import concourse.tile as tile
from concourse import bacc
from concourse.bass_utils import run_bass_kernel_spmd

B, N, T, DIN, C, R, K, L, H, OUT = 2, 1500, 20, 32, 128, 5, 5, 3, 4, 2
HD = C // H
EPS = 1e-5
NCORES = 8
NS = N // 4          # 375 real nodes per shard
NSP = NS + 1         # dest cols per core (padded even)
NSH = 384            # padded source nodes per shard (3x128)
NT = 12              # source tiles (4*384/128)
MPAD = 4 * NSH
CW = NSP // 2        # chunk width (188)
RG = [[0, 1, 2, 3], [4, 5, 6, 7]]
F32R = mybir.dt.float32r
F32 = mybir.dt.float32
BF16 = mybir.dt.bfloat16
F8 = mybir.dt.float8e4
AF = mybir.ActivationFunctionType

_NC_CACHE = {}


def _build_nc():
    if "nc" in _NC_CACHE:
        return _NC_CACHE["nc"]
    nc = bacc.Bacc(None, target_bir_lowering=False, debug=False, num_devices=NCORES)

    # ---- per-core inputs ----
    adjt = nc.dram_tensor("adjt", [2, 128, (NT // 2) * R * NSP], F8, kind="ExternalInput")
    xt = nc.dram_tensor("xt", [DIN + 1, MPAD], BF16, kind="ExternalInput")
    # ---- replicated consts (host-prelaid in SBUF layout, partition-first) ----
    wp_d = nc.dram_tensor("wp", [C, L * R * C], BF16, kind="ExternalInput")
    qkvo_d = nc.dram_tensor("qkvo", [C, L * 4 * C], BF16, kind="ExternalInput")
    w2at_d = nc.dram_tensor("w2at", [C, L * C], BF16, kind="ExternalInput")
    ow1t_d = nc.dram_tensor("ow1t", [C, C], F8, kind="ExternalInput")
    ow2t_d = nc.dram_tensor("ow2t", [C, OUT], BF16, kind="ExternalInput")
    embt_d = nc.dram_tensor("embt", [DIN + 1, C], BF16, kind="ExternalInput")
    mh_d = nc.dram_tensor("mh", [C, C], BF16, kind="ExternalInput")
    mmu_d = nc.dram_tensor("mmu", [C, C], F32R, kind="ExternalInput")
    ident_d = nc.dram_tensor("ident", [C, C], BF16, kind="ExternalInput")
    ident8_d = nc.dram_tensor("ident8", [C, C], F8, kind="ExternalInput")
    cols_d = nc.dram_tensor("cols", [C, 4 * L + 3], F32, kind="ExternalInput")
    cols2_d = nc.dram_tensor("cols2", [C, 4 * L], F32, kind="ExternalInput")
    rows_d = nc.dram_tensor("rows", [1, 4 * L * C], BF16, kind="ExternalInput")

    outt = nc.dram_tensor("outt", [OUT, NSP], F32R, kind="ExternalOutput")

    # gather buffers (internal DRAM), channel-major [C, padded shard nodes]
    g_in = [nc.dram_tensor(f"g_in_{l}", [C, NSH], F8) for l in range(2)]
    g_out = [nc.dram_tensor(f"g_out_{l}", [4 * C, NSH], F8) for l in range(2)]
    gw_in = nc.dram_tensor("gw_in", [C, 8], F8)
    gw_out = nc.dram_tensor("gw_out", [4 * C, 8], F8)

    with tile.TileContext(nc) as tc:
        with (
            tc.tile_pool(name="persist", bufs=1) as pers,
            tc.tile_pool(name="ret", bufs=2) as ret,
            tc.tile_pool(name="zwork", bufs=6) as zwork,
            tc.tile_pool(name="big", bufs=2) as big,
            tc.tile_pool(name="misc", bufs=2) as misc,
            tc.tile_pool(name="pz", bufs=3, space="PSUM") as pz,
            tc.tile_pool(name="pm", bufs=2, space="PSUM") as pm,
            tc.tile_pool(name="pp", bufs=3, space="PSUM") as pp,
        ):
            # ---------- resident tensors ----------
            adjsb = pers.tile([128, NT, R, NSP], F8, tag="adjsb")
            hnat = pers.tile([128, NT * 128], F8, tag="hnat")
            hsh = pers.tile([128, 4 * NSH], F8, tag="hsh")
            xtsb = pers.tile([DIN + 1, MPAD], BF16, tag="xtsb")
            embtsb = pers.tile([DIN + 1, C], BF16, tag="embtsb")
            wpsb = pers.tile([C, L * R * C], BF16, tag="wpsb")
            qkvosb = pers.tile([C, L * 4 * C], BF16, tag="qkvosb")
            w2atsb = pers.tile([C, L * C], BF16, tag="w2atsb")
            ow1tsb = pers.tile([C, C], F8, tag="ow1tsb")
            ow2tsb = pers.tile([C, OUT], BF16, tag="ow2tsb")
            mhsb = pers.tile([C, C], BF16, tag="mhsb")
            mmusb = pers.tile([C, C], F32R, tag="mmusb")
            identsb = pers.tile([C, C], BF16, tag="identsb")
            ident8sb = pers.tile([C, C], F8, tag="ident8sb")
            colsb = pers.tile([C, 4 * L + 3], F32, tag="colsb")
            colsb2 = pers.tile([C, 4 * L], F32, tag="colsb2")
            rowsb = pers.tile([1, 4 * L * C], BF16, tag="rowsb")
            onesb = pers.tile([1, NSP], BF16, tag="onesb")
            zerosb = pers.tile([128, NSH], F8, tag="zerosb")

            # ---------- input DMA, priority-ordered across 3 queues ----------
            # sync HWDGE: embed inputs then even adj tiles
            nc.sync.dma_start(xtsb[:], xt[:, :])
            nc.sync.dma_start(embtsb[:], embt_d[:, :])
            # scalar HWDGE: wp (needed first at L0 proj) then odd adj tiles
            nc.scalar.dma_start(wpsb[:], wp_d[:, :])
            adjflat = adjsb.rearrange("p mt r j -> p (mt r j)")
            HSZ = (NT // 2) * R * NSP
            nc.sync.dma_start(adjflat[:, 0:HSZ], adjt[0])
            nc.scalar.dma_start(adjflat[:, HSZ : 2 * HSZ], adjt[1])
            # gpsimd SWDGE: the rest, roughly in order of first use
            nc.gpsimd.dma_start(qkvosb[:], qkvo_d[:, :])
            nc.gpsimd.dma_start(mhsb[:], mh_d[:, :])
            nc.gpsimd.dma_start(mmusb[:], mmu_d[:, :])
            nc.gpsimd.dma_start(w2atsb[:], w2at_d[:, :])
            nc.gpsimd.dma_start(identsb[:], ident_d[:, :])
            nc.gpsimd.dma_start(ident8sb[:], ident8_d[:, :])
            nc.gpsimd.dma_start(colsb[:], cols_d[:, :])
            nc.gpsimd.dma_start(colsb2[:], cols2_d[:, :])
            nc.gpsimd.dma_start(rowsb[:], rows_d[:, :])
            nc.gpsimd.dma_start(ow1tsb[:], ow1t_d[:, :])
            nc.gpsimd.dma_start(ow2tsb[:], ow2t_d[:, :])

            nc.vector.memset(onesb[:], 1.0)
            nc.vector.memset(zerosb[:], 0.0)
            # tiny AllGather fired immediately: pays the first-collective
            # setup + absorbs launch skew while the adj DMA streams in
            nc.sync.dma_start(gw_in[:, :], zerosb[:, 0:8])
            nc.gpsimd.collective_compute(
                "AllGather", mybir.AluOpType.bypass,
                replica_groups=RG,
                ins=[gw_in[:, :].opt()],
                outs=[gw_out[:, :].opt()],
            )
            # pre-zero the gather inputs so pad cols (and the NSP pad col's
            # neighbors) are exact zeros, not junk DRAM that could be NaN
            nc.gpsimd.dma_start(g_in[0][:, :], zerosb[:])
            nc.gpsimd.dma_start(g_in[1][:, :], zerosb[:])

            def col(i):
                return colsb[:, i : i + 1]

            def row(l, j):
                return rowsb[0:1, (4 * l + j) * C : (4 * l + j + 1) * C]

            wp3 = wpsb.rearrange("p (l r co) -> p l r co", l=L, r=R)
            qk4 = qkvosb.rearrange("p (l i co) -> p l i co", l=L, i=4)
            w2a3 = w2atsb.rearrange("p (l co) -> p l co", l=L)

            def blip(src):
                """Tiny junk matmul chained off a chain tile: keeps the PE's
                HAM activity window busy through DVE/ACT-only stretches so the
                clock gate stays at 2.4 GHz. Output is never read."""
                jp = pz.tile([128, 8], F32, name="blip", tag="zs")
                nc.tensor.matmul(jp[:], identsb[:], src[:, 0:8],
                                 start=True, stop=True, skip_group_check=True)

            copy_eng = [0]

            def copy_alt(dst, src):
                if copy_eng[0] % 2 == 0:
                    nc.vector.tensor_copy(dst, src)
                else:
                    nc.scalar.copy(dst, src)
                copy_eng[0] += 1

            # ---------- h0 = embedding (node-major, source-node space) ----------
            for mt in range(NT):
                ep = pp.tile([128, 128], F32, tag="ps")
                nc.tensor.matmul(
                    ep[:], xtsb[:, mt * 128 : (mt + 1) * 128], embtsb[:],
                    start=True, stop=True,
                )
                copy_alt(hnat[:, mt * 128 : (mt + 1) * 128], ep[:])

            # ---------- layer machinery ----------
            CHUNKS = [(0, CW), (CW, NSP - CW)]
            RGROUPS = [(0, 1, 2), (3, 4)]

            def diff_mms(rg, c0, cw):
                """Emit the adjacency matmuls for one r-group; returns zps."""
                zps = {r: pz.tile([128, cw], F32, name=f"zps{r}", tag="zs") for r in rg}
                for mt in range(NT):
                    for r in rg:
                        nc.tensor.matmul(
                            zps[r][:],
                            hnat[:, mt * 128 : (mt + 1) * 128],
                            adjsb[:, mt, r, c0 : c0 + cw],
                            start=(mt == 0), stop=(mt == NT - 1),
                            skip_group_check=True,
                        )
                return zps

            def diff_proj(l, rg, zps, mps, cw):
                for r in rg:
                    zsb = zwork.tile([128, cw], BF16, tag="zsb")
                    copy_alt(zsb[:], zps[r][:])
                    nc.tensor.matmul(
                        mps[:], wp3[:, l, r, :], zsb[:],
                        start=(r == 0), stop=(r == R - 1),
                        skip_group_check=True,
                    )

            def diffusion(l, c0, cw):
                """Returns mps PSUM tile [128, cw] with merged diffusion."""
                mps = pm.tile([128, cw], F32, tag="mps")
                for rg in RGROUPS:
                    zps = diff_mms(rg, c0, cw)
                    diff_proj(l, rg, zps, mps, cw)
                return mps

            def ret_head(l, mps, c0, cw):
                """relu + q/k/v projections. Returns (qps, ksb, vsb)."""
                hdT = ret.tile([128, cw], BF16, tag="hdT")
                nc.scalar.activation(
                    hdT[:], mps[:], AF.Relu, bias=col(4 * l + 0), scale=1.0
                )
                qps = pp.tile([128, cw], F32, tag="ps")
                nc.tensor.matmul(qps[:], qk4[:, l, 0, :], hdT[:],
                                 start=True, stop=False, skip_group_check=True)
                nc.tensor.matmul(qps[:], row(l, 0), onesb[0:1, c0 : c0 + cw],
                                 start=False, stop=True, skip_group_check=True)
                kps = pp.tile([128, cw], F32, tag="ps")
                nc.tensor.matmul(kps[:], qk4[:, l, 1, :], hdT[:],
                                 start=True, stop=True)
                ksb = ret.tile([128, cw], F32R, tag="ksb")
                nc.scalar.activation(ksb[:], kps[:], AF.Identity, bias=row_as_col(l, 1))
                vps = pp.tile([128, cw], F32, tag="ps")
                nc.tensor.matmul(vps[:], qk4[:, l, 2, :], hdT[:],
                                 start=True, stop=True)
                vsb = ret.tile([128, cw], F32R, tag="vsb")
                nc.vector.tensor_scalar_add(vsb[:], vps[:], row_as_col(l, 2))
                return qps, ksb, vsb

            def row_as_col(l, j):
                # kb/vb applied as per-partition activation-bias columns
                return colsb2[:, (4 * l + j) : (4 * l + j) + 1]

            def ret_tail(l, qps, ksb, vsb, c0, cw, hnT_full):
                """Retention tail as 4 emission segments so the caller can
                interleave them with another chunk's diffusion matmuls (engine
                streams execute in emission order)."""
                st = {}

                def seg1():
                    st["qk"] = ret.tile([128, cw], BF16, name="qk", tag="qk")
                    nc.vector.tensor_mul(st["qk"][:], ksb[:], qps[:])
                    st["sbps"] = pp.tile([128, cw], F32, name="sbps", tag="ps")
                    nc.tensor.matmul(st["sbps"][:], mhsb[:], st["qk"][:],
                                     start=True, stop=True)
                    st["osb"] = ret.tile([128, cw], BF16, name="osb", tag="osb")
                    nc.vector.tensor_mul(st["osb"][:], vsb[:], st["sbps"][:])
                    blip(st["qk"])

                def seg2():
                    st["o2ps"] = pp.tile([128, cw], F32, name="o2ps", tag="ps")
                    nc.tensor.matmul(st["o2ps"][:], qk4[:, l, 3, :], st["osb"][:],
                                     start=True, stop=False, skip_group_check=True)
                    nc.tensor.matmul(st["o2ps"][:], row(l, 3),
                                     onesb[0:1, c0 : c0 + cw],
                                     start=False, stop=True, skip_group_check=True)
                    st["sq"] = ret.tile([128, cw], F32R, name="sq", tag="sq")
                    nc.scalar.activation(st["sq"][:], st["o2ps"][:], AF.Square)
                    st["o2sb"] = ret.tile([128, cw], F32R, name="o2sb", tag="o2sb")
                    nc.vector.tensor_copy(st["o2sb"][:], st["o2ps"][:])

                def seg3():
                    mups = pp.tile([128, cw], F32, tag="ps")
                    nc.tensor.matmul(mups[:], mmusb[:], st["o2sb"][:],
                                     start=True, stop=True)
                    msps = pp.tile([128, cw], F32, tag="ps")
                    nc.tensor.matmul(msps[:], mmusb[:], st["sq"][:],
                                     start=True, stop=True)
                    mu2 = ret.tile([128, cw], F32R, tag="mu2")
                    nc.scalar.activation(mu2[:], mups[:], AF.Square)
                    tsb = ret.tile([128, cw], BF16, tag="tsb")
                    nc.vector.tensor_sub(tsb[:], st["o2sb"][:], mups[:])
                    varsb = ret.tile([128, cw], F32R, tag="varsb")
                    nc.vector.tensor_sub(varsb[:], msps[:], mu2[:])
                    rstd = ret.tile([128, cw], BF16, tag="rstd")
                    # 1/sqrt(var+eps) in one table-resident activation; the
                    # abs is a no-op since var+eps > 0
                    nc.scalar.activation(rstd[:], varsb[:],
                                         AF.Abs_reciprocal_sqrt, bias=col(4 * L))
                    blip(tsb)
                    # hr = (o2-mu)*rstd*gn_g + gn_b; the gn_b term is folded
                    # into the w2 bias on the host, so one stt does the rest
                    st["hrT"] = ret.tile([128, cw], BF16, name="hrT", tag="hrT")
                    nc.vector.scalar_tensor_tensor(
                        st["hrT"][:], tsb[:], col(4 * l + 2), rstd[:],
                        mybir.AluOpType.mult, mybir.AluOpType.mult,
                    )

                def seg4():
                    h2ps = pp.tile([128, cw], F32, tag="ps")
                    nc.tensor.matmul(h2ps[:], w2a3[:, l, :], st["hrT"][:],
                                     start=True, stop=True)
                    nc.scalar.activation(
                        hnT_full[:, c0 : c0 + cw], h2ps[:], AF.Relu,
                        bias=col(4 * l + 1), scale=1.0,
                    )

                return seg1, seg2, seg3, seg4

            # ---------- layers ----------
            for l in range(L):
                if l > 0:
                    # rebuild hnat from the gathered channel-major h
                    for s, eng in enumerate((nc.sync, nc.scalar, nc.gpsimd,
                                             nc.sync)):
                        eng.dma_start(
                            hsh[:, s * NSH : (s + 1) * NSH],
                            g_out[l - 1][s * 128 : (s + 1) * 128, :],
                        )
                    for t in range(NT):
                        # fp8 PE transpose requires output element step 2
                        tp = pp.tile([128, 256], F8, tag="ps")
                        nc.tensor.transpose(
                            tp[:, 0:256:2], hsh[:, t * 128 : (t + 1) * 128],
                            ident8sb[:],
                        )
                        copy_alt(hnat[:, t * 128 : (t + 1) * 128], tp[:, 0:256:2])

                hnT_full = big.tile([C, NSP], F8, tag="hnT")
                (a0, aw), (b0, bw) = CHUNKS

                mpsA = diffusion(l, a0, aw)
                qA = ret_head(l, mpsA, a0, aw)
                # chunk B diffusion interleaved with chunk A retention tail:
                # A's DVE/ACT chain runs while the PE grinds B's adjacency
                # matmuls; A's few PE hops slot between B's r-groups.
                s1, s2, s3, s4 = ret_tail(l, *qA, a0, aw, hnT_full)
                mpsB = pm.tile([128, bw], F32, tag="mps")
                zB1 = diff_mms(RGROUPS[0], b0, bw)
                s1()
                diff_proj(l, RGROUPS[0], zB1, mpsB, bw)
                s2()
                zB2 = diff_mms(RGROUPS[1], b0, bw)
                s3()
                diff_proj(l, RGROUPS[1], zB2, mpsB, bw)
                s4()
                qB = ret_head(l, mpsB, b0, bw)
                t1, t2, t3, t4 = ret_tail(l, *qB, b0, bw, hnT_full)
                t1(); t2(); t3(); t4()

                if l < 2:
                    nc.sync.dma_start(g_in[l][:, 0:NSP], hnT_full[:, 0:NSP])
                    nc.gpsimd.collective_compute(
                        "AllGather", mybir.AluOpType.bypass,
                        replica_groups=RG,
                        ins=[g_in[l][:, :].opt()],
                        outs=[g_out[l][:, :].opt()],
                    )
                    # self-paced junk mm/copy hops to keep the PE clock gate
                    # warm across the collective wait (~1us per hop)
                    cur = hnT_full
                    for i in range(5):
                        jp = pz.tile([128, 8], F32, name=f"wc{i}", tag="zs")
                        nc.tensor.matmul(jp[:], identsb[:], cur[:, 0:8],
                                         start=True, stop=True,
                                         skip_group_check=True)
                        js = zwork.tile([128, 8], BF16, name=f"wcs{i}", tag="zsb")
                        nc.scalar.copy(js[:], jp[:])
                        cur = js
                else:
                    # final head
                    hmps = pp.tile([128, NSP], F32, tag="ps")
                    nc.tensor.matmul(hmps[:], ow1tsb[:], hnT_full[:],
                                     start=True, stop=True)
                    hmsb = misc.tile([C, NSP], BF16, tag="hmsb")
                    nc.scalar.activation(
                        hmsb[:], hmps[:], AF.Relu, bias=col(4 * L + 1)
                    )
                    oops = pp.tile([OUT, NSP], F32, tag="ps")
                    nc.tensor.matmul(oops[:], ow2tsb[:], hmsb[:],
                                     start=True, stop=True)
                    oosb = misc.tile([OUT, NSP], F32R, tag="oosb")
                    nc.scalar.activation(
                        oosb[:], oops[:], AF.Identity,
                        bias=colsb[0:OUT, 4 * L + 2 : 4 * L + 3],
                    )
                    nc.sync.dma_start(outt[:, :], oosb[:])

    nc.finalize()
    _NC_CACHE["nc"] = nc
    return nc


def _prep(inputs):
    import ml_dtypes

    bf16 = ml_dtypes.bfloat16
    f8 = ml_dtypes.float8_e4m3
    f32 = np.float32

    def g(name):
        return np.asarray(inputs[name], f32)

    x, adj = g("x"), g("adj_list")
    alpha, transition = g("alpha"), g("transition")
    conv_w, conv_b = g("conv_w"), g("conv_b")
    w1, b1, eb1 = g("w1"), g("b1"), g("eb1")
    w2, b2, eb2 = g("w2"), g("b2"), g("eb2")

    a = alpha - alpha.max(-1, keepdims=True)
    e = np.exp(a)
    srow = (e / e.sum(-1, keepdims=True)).sum(-1)          # [L,R]
    Wm = transition.mean(axis=2)                            # [L,R,C,C]
    Wp = (conv_w * srow)[:, :, None, None] * np.swapaxes(Wm, -1, -2)

    hp = np.zeros((C,), f32)
    b2eff = np.zeros((L, C), f32)
    for l in range(L):
        # gn_b's contribution through w2 is folded in here so the kernel's
        # GN affine is a single (x-mu)*rstd*gn_g op
        b2eff[l] = b2[l] + eb2[l] + w2[l][:, C:] @ hp + w2[l][:, :C] @ g("gn_b")[l]
        hp = np.maximum(hp @ w1[l].T + b1[l] + eb1[l], 0.0).astype(f32)

    qkvo = np.stack(
        [np.swapaxes(g(w), -1, -2) for w in ("qw", "kw", "vw", "ow")], axis=1
    )  # [L,4,C,C] in lhsT layout

    hid = np.arange(C) // HD
    same = (hid[:, None] == hid[None, :]).astype(f32)       # [C,C]

    cols = np.zeros((C, 4 * L + 3), f32)
    cols2 = np.zeros((C, 4 * L), f32)
    rows = np.zeros((1, 4 * L * C), f32)
    for l in range(L):
        cols[:, 4 * l + 0] = conv_b[l]
        cols[:, 4 * l + 1] = b2eff[l]
        cols[:, 4 * l + 2] = g("gn_g")[l]
        cols[:, 4 * l + 3] = g("gn_b")[l]
        for j, nm in enumerate(("qb", "kb", "vb", "ob")):
            cols2[:, 4 * l + j] = g(nm)[l]
            rows[0, (4 * l + j) * C : (4 * l + j + 1) * C] = g(nm)[l]
    cols[:, 4 * L] = EPS
    cols[:, 4 * L + 1] = g("out_b1")
    cols[:OUT, 4 * L + 2] = g("out_b2")

    consts = {
        "wp": np.ascontiguousarray(
            (Wp / 16.0).transpose(2, 0, 1, 3).reshape(C, L * R * C)
        ).astype(bf16),
        "qkvo": np.ascontiguousarray(
            qkvo.transpose(2, 0, 1, 3).reshape(C, L * 4 * C)
        ).astype(bf16),
        "w2at": np.ascontiguousarray(
            np.swapaxes(w2[:, :, :C], -1, -2).transpose(1, 0, 2).reshape(C, L * C)
        ).astype(bf16),
        "ow1t": np.ascontiguousarray(g("out_w1").T).astype(f8),
        "ow2t": np.ascontiguousarray(g("out_w2").T).astype(bf16),
        "embt": np.concatenate(
            [g("emb_w").T, g("emb_b")[None, :]], axis=0
        ).astype(bf16),
        "mh": same.astype(bf16),
        "mmu": (same / HD).astype(f32),
        "ident": np.eye(C, dtype=f32).astype(bf16),
        "ident8": np.eye(C, dtype=f32).astype(f8),
        "cols": cols,
        "cols2": cols2,
        "rows": rows.astype(bf16),
    }

    xlast = x[:, :, -1, :]                                   # [B,N,DIN]
    in_maps = []
    for k in range(NCORES):
        b, s = k // 4, k % 4
        asub = adj[b][:, s * NS : (s + 1) * NS, :]           # [R,NS,N] dest rows
        ap = np.zeros((R, NSP, MPAD), f32)
        for s2 in range(4):
            ap[:, :NS, s2 * NSH : s2 * NSH + NS] = asub[:, :, s2 * NS : (s2 + 1) * NS]
        a3 = ap.transpose(2, 0, 1).reshape(NT, 128, R, NSP)  # [mt, mi, R, NSP]
        a3 = (
            a3.reshape(2, NT // 2, 128, R * NSP)
            .transpose(0, 2, 1, 3)                           # [half, mi, 6, R*NSP]
            .reshape(2, 128, (NT // 2) * R * NSP)
        )
        xt = np.zeros((DIN + 1, MPAD), f32)
        for s2 in range(4):
            xt[:DIN, s2 * NSH : s2 * NSH + NS] = xlast[b, s2 * NS : (s2 + 1) * NS].T
        xt[DIN, :] = 1.0
        in_maps.append(
            dict(consts, adjt=(np.ascontiguousarray(a3) * 16.0).astype(f8),
                 xt=xt.astype(bf16))
        )
    return in_maps


def kernel(**inputs):
    nc = _build_nc()
    in_maps = _prep(inputs)
    res = run_bass_kernel_spmd(nc, in_maps, core_ids=list(range(NCORES)))
    out = np.zeros((B, N, OUT), np.float32)
    for k in range(NCORES):
        b, s = k // 4, k % 4
        out[b, s * NS : (s + 1) * NS, :] = res.results[k]["outt"][:, :NS].T
    return out
